# revision 87
# baseline (speedup 1.0000x reference)
"""Trainium2 Bass kernel for GQA attention (B=2, S=2048, D=4096, 32 q-heads,
8 kv-heads, head_dim=128, RoPE, causal) distributed over 8 NeuronCores.

Sharding: tensor-parallel over heads for QKV+attention (core c owns q-heads
4c..4c+3 and kv-head c, all sequence positions), then per-head AllToAlls
re-shard the attention output from head-sharded to row-sharded (overlapped
with attention of later heads) so the output projection wo contracts locally;
final output rows are gathered on the host.

Device dataflow per core (identical program on all 8 cores, data differs):
  - stream x^T tiles once; accumulate K^T, V^T (own kv head) and Q^T (4 own
    heads) in PSUM; drain PSUM quickly via ScalarE copies; rotate-half RoPE
    on VectorE (weights are column-permuted on the host so RoPE pairs are
    (i, i+64) within each head); V^T -> V via PE transposes, inline per chunk.
  - causal attention per (head, batch, q-tile of 128), software-pipelined:
    scores in PSUM (PE) -> exp on ScalarE (accumulated row-sum = softmax
    denominator for free; no max subtraction needed at these magnitudes) ->
    normalize P by 1/den (VectorE) -> PE-transpose P -> P^T @ V on PE.
  - AllToAll per head (head-sharded -> row-sharded), overlapped.
  - out^T = wo^T @ attn_out^T for this core's 512 rows.
Host returns out[rows_c, :] = out_c^T.T concatenated over cores.
"""
import sys
sys.path.insert(0, "/opt/trn_rl_repo")
import math
import numpy as np

import concourse.bass as bass
import concourse.bacc as bacc
import concourse.tile as tile
import concourse.mybir as mybir
from concourse.bass_utils import run_bass_kernel_spmd
from concourse.dt import dt

B, S, D = 2, 2048, 4096
HQ, HKV, HD = 32, 8, 128
NC_ = 8                       # cores
ROWS = B * S                  # 4096
RPC = ROWS // NC_             # 512 rows per core
HPC = HQ // NC_               # 4 q-heads per core
QT = S // HD                  # 16 q-tiles per batch
DT_ = 32                      # d-tiles (D/128)
SCALE = 1.0 / math.sqrt(HD)
THETA = 10000.0

F32 = mybir.dt.float32
CDT = mybir.dt.bfloat16       # compute dtype for matmul operands
NP_CDT = dt.np(CDT)

_CACHE = {}


def _build():
    nc = bacc.Bacc("TRN2", target_bir_lowering=False, debug=False,
                   num_devices=NC_)
    AF = mybir.ActivationFunctionType

    # ---- dram I/O (names = in_maps keys) ----
    xT = nc.dram_tensor("xT", [DT_, 128, ROWS], CDT, kind="ExternalInput")
    wq = nc.dram_tensor("wq", [128, DT_ * HPC * HD], CDT, kind="ExternalInput")
    wk = nc.dram_tensor("wk", [128, DT_ * HD], CDT, kind="ExternalInput")
    wv = nc.dram_tensor("wv", [128, DT_ * HD], CDT, kind="ExternalInput")
    wo = nc.dram_tensor("wo", [(DT_ // 4) * HPC, 128, NC_ * 4 * HD], CDT,
                        kind="ExternalInput")
    csa = nc.dram_tensor("csa", [128, S], F32, kind="ExternalInput")  # cos|cos
    csb = nc.dram_tensor("csb", [128, S], F32, kind="ExternalInput")  # -sin|sin
    msk = nc.dram_tensor("msk", [128, HD], CDT, kind="ExternalInput")
    idn = nc.dram_tensor("idn", [128, 128], CDT, kind="ExternalInput")
    out = nc.dram_tensor("out", [DT_ * 128, RPC], F32, kind="ExternalOutput")

    with tile.TileContext(nc) as tc:
        with tc.tile_pool(name="const", bufs=1) as constp, \
             tc.tile_pool(name="dram", bufs=1, space="DRAM") as dram:
            # constants (cos/sin table loads are emitted after chunk 0's
            # scalar-queue weight loads; first rope use is ~45us in)
            csa_sb = constp.tile([128, S], F32)
            csb_sb = constp.tile([128, S], F32)
            msk_sb = constp.tile([128, HD], CDT)
            nc.scalar.dma_start(msk_sb[:], msk[:])
            idn_sb = constp.tile([128, 128], CDT)
            nc.scalar.dma_start(idn_sb[:], idn[:])
            # staging for head-0's a2a output slices: lives in never-released
            # SBUF so its loads run as soon as head 0's collective lands
            # (~150us before phase O needs them), with no WAR on k/q/v
            a2h0 = constp.tile([128, NC_ * RPC], CDT)

            # tiny warmup AllToAll: absorbs the one-time collective setup
            # cost (~120us on the first collective) under phase P's matmuls
            wu_in = dram.tile([NC_, 64], F32)
            wu_out = dram.tile([NC_, 64], F32)
            nc.gpsimd.collective_compute(
                "AllToAll", mybir.AluOpType.bypass,
                ins=[wu_in[:].opt()], outs=[wu_out[:].opt()],
                replica_groups=[list(range(NC_))],
            )

            # wo staging pool opened before persist (manual scoping: persist
            # closes first, wos survives into phase O) so the first wo tiles
            # can prefetch during the last attention head
            wos_cm = tc.tile_pool(name="wos", bufs=3)
            wos = wos_cm.__enter__()

            # persistent activations (pool closed before phase O so its SBUF
            # is reclaimed for the f32 partial-output buffer)
            persist_cm = tc.tile_pool(name="persist", bufs=1)
            persist = persist_cm.__enter__()
            k_sb = persist.tile([128, ROWS], CDT)           # K^T (rope'd)
            vn_sb = persist.tile([128, ROWS], CDT)          # V natural tiles
            q_sb = persist.tile([128, HPC * ROWS], CDT)     # Q^T per head
            ao_sb = persist.tile([128, HPC * ROWS], CDT)    # attn_out^T per head

            def _rope(dst, src_sb, swp_ps, pos0, n, tmp_pool):
                """dst[128, n] (SBUF CDT) = rope(src_sb[128, n] SBUF CDT),
                given swp_ps[128, n] (PSUM f32) = half-swapped src (from a PE
                matmul with the swap permutation). Positions pos0.. (one
                batch). All ops are full-tile, partition-aligned:
                  dst = src * [cos|cos] + swapped(src) * [-sin|sin]."""
                ca = csa_sb[:, pos0:pos0 + n]
                cb = csb_sb[:, pos0:pos0 + n]
                t = tmp_pool.tile([128, n], F32, tag="ropetmp")
                nc.vector.tensor_mul(t[:], src_sb[:], ca)
                u = tmp_pool.tile([128, n], F32, tag="ropetmp2")
                nc.vector.tensor_mul(u[:], swp_ps[:], cb)
                nc.vector.tensor_add(dst[:], t[:], u[:])

            # ---------------- phase P: projections ----------------
            with tc.tile_pool(name="wts", bufs=1) as wtp, \
                 tc.tile_pool(name="xs", bufs=6) as xsp, \
                 tc.tile_pool(name="ptmp", bufs=2) as ptmp, \
                 tc.tile_pool(name="drain", bufs=2) as drp, \
                 tc.tile_pool(name="pps", bufs=1, space="PSUM") as pps, \
                 tc.tile_pool(name="kpp", bufs=2, space="PSUM") as kpp, \
                 tc.tile_pool(name="miscp", bufs=1, space="PSUM") as miscp:
                # weight loads chunked (8 chunks of 4 d-tiles); chunk 0 lands
                # first so matmuls start immediately, the rest interleave
                # with the first row-chunk's xT stream
                wk_sb = wtp.tile([128, DT_ * HD], CDT)
                wv_sb = wtp.tile([128, DT_ * HD], CDT)
                wq_sb = wtp.tile([128, DT_ * HPC * HD], CDT)

                def load_wchunk(qd):
                    # chunk 0's wv/wq ride the scalar queue so the first
                    # d-iterations' weights land in parallel with wk/xt on
                    # sync; later chunks stay on sync (scalar carries the
                    # cos/sin tables then drains)
                    weng = nc.scalar if qd == 0 else nc.sync
                    c0, c1 = qd * DT_ * HD // 8, (qd + 1) * DT_ * HD // 8
                    nc.sync.dma_start(wk_sb[:, c0:c1], wk[:, c0:c1])
                    weng.dma_start(wv_sb[:, c0:c1], wv[:, c0:c1])
                    q0_, q1_ = qd * DT_ * HPC * HD // 8, \
                        (qd + 1) * DT_ * HPC * HD // 8
                    if qd == 0:
                        # d0's four head-slices first: the long wq transfer
                        # otherwise gates the first d-iteration by ~5us
                        nc.scalar.dma_start(wq_sb[:, 0:HPC * HD],
                                            wq[:, 0:HPC * HD])
                        nc.scalar.dma_start(wq_sb[:, HPC * HD:q1_],
                                            wq[:, HPC * HD:q1_])
                    else:
                        weng.dma_start(wq_sb[:, q0_:q1_], wq[:, q0_:q1_])

                # first x tile ahead of the bulk weight loads so the first
                # matmul's operands land as early as possible
                # throwaway warm-up DMAs: the first completion on a cold
                # queue pays ~5us of semaphore/pipeline latency -- absorb it
                # on dummies so xt00/wk0 complete promptly
                wrm = xsp.tile([128, 512], CDT, tag="xt", name="dma_wrm")
                nc.sync.dma_start(wrm[:, 0:4], xT[0, :, 0:4])
                nc.scalar.dma_start(wrm[:, 4:8], xT[0, :, 4:8])
                xt00 = xsp.tile([128, 512], CDT, tag="xt", name="xt_pre")
                nc.sync.dma_start(xt00[:], xT[0, :, 0:512])
                load_wchunk(0)
                nc.scalar.dma_start(csa_sb[:], csa[:])
                nc.scalar.dma_start(csb_sb[:], csb[:])

                NCH = ROWS // 512   # 8 row-chunks
                for ch in range(NCH):
                    pos0 = (ch * 512) % S
                    kp = kpp.tile([128, 512], F32, tag="kp")
                    vp = pps.tile([128, 512], F32, tag="vp")
                    qp = [pps.tile([128, 512], F32, tag=f"qp{h}",
                                   name=f"qp{h}_{ch}")
                          for h in range(HPC)]
                    for d in range(DT_):
                        if ch == 0 and d == 0:
                            xt = xt00
                        else:
                            xt = xsp.tile([128, 512], CDT, tag="xt")
                            nc.sync.dma_start(
                                xt[:], xT[d, :, ch * 512:(ch + 1) * 512])
                        if ch == 0 and d % 4 == 0 and d // 4 + 1 < 8:
                            load_wchunk(d // 4 + 1)
                        st, sp = (d == 0), (d == DT_ - 1)
                        # order matches drain completion: kp double-buffered
                        # (never waits), qp2/qp3 drained on DVE, vp/qp0/qp1
                        # on ACT
                        nc.tensor.matmul(kp[:], wk_sb[:, d * HD:(d + 1) * HD],
                                         xt[:], start=st, stop=sp)
                        for h in (2, 3):
                            w0 = (d * HPC + h) * HD
                            nc.tensor.matmul(qp[h][:],
                                             wq_sb[:, w0:w0 + HD],
                                             xt[:], start=st, stop=sp)
                        nc.tensor.matmul(vp[:], wv_sb[:, d * HD:(d + 1) * HD],
                                         xt[:], start=st, stop=sp)
                        for h in (0, 1):
                            w0 = (d * HPC + h) * HD
                            nc.tensor.matmul(qp[h][:],
                                             wq_sb[:, w0:w0 + HD],
                                             xt[:], start=st, stop=sp)
                    sl = slice(ch * 512, (ch + 1) * 512)
                    # drain PSUM fast (cast to bf16): vp/qp0/qp1 on ACT,
                    # qp2/qp3 on DVE, kf last on ACT (kp is double-buffered
                    # so its drain only gates the rope, not the next chunk)
                    qf = [drp.tile([128, 512], CDT, tag=f"qf{h}",
                                   name=f"qf{h}_{ch}")
                          for h in range(HPC)]
                    vf = drp.tile([128, 512], CDT, tag="vf")
                    nc.scalar.copy(vf[:], vp[:])   # V^T chunk (bf16)
                    nc.vector.tensor_copy(qf[2][:], qp[2][:])
                    nc.vector.tensor_copy(qf[3][:], qp[3][:])
                    nc.scalar.copy(qf[0][:], qp[0][:])
                    nc.scalar.copy(qf[1][:], qp[1][:])
                    kf = drp.tile([128, 512], CDT, tag="kf")
                    nc.scalar.copy(kf[:], kp[:])
                    # rotate-half via partition-offset DMA copies on the SP
                    # HWDGE ring instead of PE permutation matmuls: removes
                    # 5 N=512 matmuls/chunk from the 99%-busy PE stream.
                    # (Issuing from the ACT queue instead stalls ScalarE's
                    # PSUM drains; the SP queue only carries DMAs here and
                    # the xt prefetch depth absorbs the extra transfers.)
                    def _swap(src, nm):
                        sw = drp.tile([128, 512], CDT, tag="swp",
                                      name=f"swp_{ch}_{nm}")
                        nc.sync.dma_start(sw[0:64, :], src[64:128, :])
                        nc.sync.dma_start(sw[64:128, :], src[0:64, :])
                        return sw
                    _rope(k_sb[:, sl], kf[:], _swap(kf[:], "k"), pos0, 512,
                          ptmp)
                    for h in range(HPC):
                        _rope(q_sb[:, h * ROWS + ch * 512:
                                   h * ROWS + (ch + 1) * 512],
                              qf[h][:], _swap(qf[h][:], f"q{h}"), pos0, 512,
                              ptmp)
                    # V^T -> V natural, inline (PE transposes + DVE drain)
                    vt = miscp.tile([128, 512], CDT, tag="misc",
                                    name=f"vt_{ch}")
                    for t in range(4):
                        nc.tensor.transpose(vt[:, t * 128:(t + 1) * 128],
                                            vf[:, t * 128:(t + 1) * 128],
                                            idn_sb[:])
                    nc.vector.tensor_copy(vn_sb[:, sl], vt[:])

            # ---------------- phase A: attention (+ per-head A2A) --------
            # separate DRAM tiles per head: DRAM dep-tracking is tensor-
            # granular, so a shared buffer makes head h+1's stores falsely
            # wait on collective h's reads (serializing the A2A pipeline)
            a2a_in = [dram.tile([NC_, 128, RPC], CDT, name=f"a2ai{h}")
                      for h in range(HPC)]
            a2a_out = [dram.tile([NC_, 128, RPC], CDT, name=f"a2ao{h}")
                       for h in range(HPC)]

            with tc.tile_pool(name="att", bufs=3) as att, \
                 tc.tile_pool(name="attd", bufs=6) as attd, \
                 tc.tile_pool(name="sps", bufs=2, space="PSUM") as sps, \
                 tc.tile_pool(name="tps2", bufs=2, space="PSUM") as tps2, \
                 tc.tile_pool(name="ops", bufs=2, space="PSUM") as ops:

                def stage1(h, b, j):
                    """QK -> exp -> den/normalize -> xbar-transposed P^T."""
                    klen = HD * (j + 1)
                    q0 = h * ROWS + b * S + j * HD
                    qt_ap = q_sb[:, q0:q0 + HD]
                    nkc = (klen + 1023) // 1024
                    p_t = att.tile([128, 2048], CDT, tag="p",
                                   name=f"p_{h}_{b}_{j}")
                    den = attd.tile([128, 4], F32, tag="den",
                                    name=f"den_{h}_{b}_{j}")
                    for kc in range(nkc):
                        k0 = kc * 1024
                        kl = min(1024, klen - k0)
                        sp_ = sps.tile([128, 1024], F32, tag="sp",
                                       name=f"sp_{h}_{b}_{j}_{kc}")
                        for nn in range(0, kl, 512):
                            nw = min(512, kl - nn)
                            if k0 + nn + nw == klen:
                                # final block: causal mask accumulated on PE
                                # via mask^T @ I (PE has slack once the P
                                # transposes move to the DMA crossbar)
                                if nw > HD:
                                    nc.tensor.matmul(
                                        sp_[:, nn:nn + nw - HD], qt_ap,
                                        k_sb[:, b * S + k0 + nn:
                                             b * S + k0 + nn + nw - HD],
                                        start=True, stop=True)
                                d0 = nn + nw - HD
                                nc.tensor.matmul(
                                    sp_[:, d0:d0 + HD], qt_ap,
                                    k_sb[:, b * S + klen - HD:b * S + klen],
                                    start=True, stop=False)
                                nc.tensor.matmul(
                                    sp_[:, d0:d0 + HD], msk_sb[:], idn_sb[:],
                                    start=False, stop=True)
                            else:
                                nc.tensor.matmul(
                                    sp_[:, nn:nn + nw], qt_ap,
                                    k_sb[:, b * S + k0 + nn:
                                         b * S + k0 + nn + nw],
                                    start=True, stop=True)
                        nc.scalar.activation(
                            p_t[:, k0:k0 + kl], sp_[:, 0:kl],
                            AF.Exp, scale=SCALE,
                            accum_out=den[:, kc:kc + 1])
                    for kc in range(1, nkc):
                        nc.vector.tensor_add(den[:, 0:1], den[:, 0:1],
                                             den[:, kc:kc + 1])
                    rden = attd.tile([128, 1], F32, tag="rden",
                                     name=f"rden_{h}_{b}_{j}")
                    nc.vector.reciprocal(rden[:], den[:, 0:1])
                    nc.vector.tensor_scalar_mul(p_t[:, 0:klen],
                                                p_t[:, 0:klen], rden[:])
                    return p_t

                def stage2a(h, b, j, p_t):
                    """P transpose (PE) + PSUM->SBUF copies (DVE)."""
                    pt_t = att.tile([128, 2048], CDT, tag="pt",
                                    name=f"pt_{h}_{b}_{j}")
                    for g in range(0, j + 1, 4):
                        gw = min(4, j + 1 - g)
                        tp = tps2.tile([128, 512], CDT, tag="tp",
                                       name=f"tp_{h}_{b}_{j}_{g}")
                        for t in range(gw):
                            c0 = (g + t) * HD
                            nc.tensor.transpose(tp[:, t * HD:(t + 1) * HD],
                                                p_t[:, c0:c0 + HD], idn_sb[:])
                        nc.vector.tensor_copy(pt_t[:, g * HD:(g + gw) * HD],
                                              tp[:, 0:gw * HD])
                    return pt_t

                def stage2b(h, b, j, pt_t, otg):
                    """PV accumulate -> drain OT group. Runs one tile behind
                    stage2a so the PV never waits on the fresh P^T copy."""
                    jj = j % 4
                    for kt in range(j + 1):
                        nc.tensor.matmul(
                            otg[:, jj * HD:(jj + 1) * HD],
                            vn_sb[:, (b * QT + kt) * HD:(b * QT + kt + 1) * HD],
                            pt_t[:, kt * HD:(kt + 1) * HD],
                            start=(kt == 0), stop=(kt == j))
                    if jj == 0:
                        # group complete (descending j): drain 4 OTs at once
                        g0 = h * ROWS + b * S + j * HD
                        nc.vector.tensor_copy(
                            ao_sb[:, g0:g0 + 4 * HD], otg[:])

                wo_pre = {}
                for h in range(HPC):
                    if h == HPC - 1:
                        # prefetch the first two phase-O weight tiles while
                        # the sync queue is otherwise idle
                        for g in (0, 1):
                            wt = wos.tile([128, NC_ * 4 * HD], CDT, tag="wo",
                                          name=f"wo_{g}_0")
                            nc.sync.dma_start(wt[:], wo[g * HPC, :, :])
                            wo_pre[(g, 0)] = wt
                    pend_a = []
                    pend_b = []
                    otg = [None]

                    def run_b(args):
                        hh, bb, jj_, pt_t = args
                        if jj_ % 4 == 3:
                            otg[0] = ops.tile([128, 512], F32, tag="ot",
                                              name=f"ot_{hh}_{bb}_{jj_}")
                        stage2b(hh, bb, jj_, pt_t, otg[0])

                    # q-tiles in descending j: the big tiles lead (hiding the
                    # exp/normalize latency at pipeline fill), the tiny ones
                    # flush the tail just before this head's AllToAll
                    for b in range(B):
                        for j in range(QT - 1, -1, -1):
                            p_t = stage1(h, b, j)
                            pend_a.append((h, b, j, p_t))
                            if len(pend_a) > 2:
                                ha, ba, ja, pa = pend_a.pop(0)
                                pend_b.append((ha, ba, ja,
                                               stage2a(ha, ba, ja, pa)))
                            if len(pend_b) > 1:
                                run_b(pend_b.pop(0))
                    for ha, ba, ja, pa in pend_a:
                        pend_b.append((ha, ba, ja, stage2a(ha, ba, ja, pa)))
                    for args in pend_b:
                        run_b(args)
                    # per-head AllToAll, overlaps later heads' attention;
                    # stores on sync+gpsimd only -- NEVER the scalar queue
                    # (a blocked store there stalls the in-order exp stream)
                    for r in range(NC_):
                        seng = nc.sync if r % 2 == 0 else nc.gpsimd
                        seng.dma_start(
                            a2a_in[h][r, :, :],
                            ao_sb[:, h * ROWS + r * RPC:
                                  h * ROWS + (r + 1) * RPC])
                    nc.gpsimd.collective_compute(
                        "AllToAll", mybir.AluOpType.bypass,
                        ins=[a2a_in[h][:].opt()], outs=[a2a_out[h][:].opt()],
                        replica_groups=[list(range(NC_))],
                    )

            persist_cm.__exit__(None, None, None)

            # ---------------- phase O: output projection ----------------
            # h-MAJOR passes with SBUF-accumulated partials: head h's A2A is
            # only needed ~67us*h into phase O, so even ~100us of inter-core
            # arrival skew on the collectives stays off the critical path.
            # PSUM accumulates over sources s within one (g,h) pass; the
            # cross-h accumulation runs on the otherwise-idle DVE in SBUF.
            NG = DT_ // 4   # 8 od-groups of 4 PSUM banks, double-buffered
            with tc.tile_pool(name="osb", bufs=1) as osb, \
                 tc.tile_pool(name="opp", bufs=2, space="PSUM") as opp:
                # NOTE: each tile allocation must immediately precede its
                # first writer — the released-persist-zone dependency is
                # attached to the next emitted instruction, so reordering
                # allocations against the load/sweep emission corrupts data
                # (measured: NaN / 1e-1 errors with po allocated early).
                ao2 = osb.tile([128, DT_ * RPC], CDT)
                # ALL a2a-output loads stay on the gpsimd queue: they wait on
                # collective completion, and the scheduler may hoist them —
                # on sync/scalar such a blocked DMA stalls the exp stream and
                # the whole machine (measured 4-12us global quiesce)
                for h in range(HPC):
                    for s_ in range(NC_):
                        ct = s_ * HPC + h
                        if h == 0:
                            # head 0 -> the early-loadable constp staging
                            nc.gpsimd.dma_start(
                                a2h0[:, s_ * RPC:(s_ + 1) * RPC],
                                a2a_out[h][s_, :, :])
                        else:
                            nc.gpsimd.dma_start(
                                ao2[:, ct * RPC:(ct + 1) * RPC],
                                a2a_out[h][s_, :, :])
                po = osb.tile([128, DT_ * RPC], F32)   # partial out, 32 od
                for h in range(HPC):
                    for g in range(NG):
                        ops_ = opp.tile([128, 4 * RPC], F32, tag="op",
                                        name=f"op_{g}_{h}")
                        if (g, h) in wo_pre:
                            wo_sb = wo_pre.pop((g, h))
                        else:
                            wo_sb = wos.tile([128, NC_ * 4 * HD], CDT,
                                             tag="wo", name=f"wo_{g}_{h}")
                            # alternate queues: halves load latency and keeps
                            # the first passes off the a2a-store backlog
                            weng = nc.sync if (h * NG + g) % 2 == 0 \
                                else nc.scalar
                            weng.dma_start(wo_sb[:], wo[g * HPC + h, :, :])
                        for s_ in range(NC_):
                            ct = s_ * HPC + h
                            rhs = (a2h0[:, s_ * RPC:(s_ + 1) * RPC]
                                   if h == 0 else
                                   ao2[:, ct * RPC:(ct + 1) * RPC])
                            for i in range(4):
                                w0 = (s_ * 4 + i) * HD
                                nc.tensor.matmul(
                                    ops_[:, i * RPC:(i + 1) * RPC],
                                    wo_sb[:, w0:w0 + HD], rhs,
                                    start=(s_ == 0), stop=(s_ == NC_ - 1))
                        for i in range(4):
                            od = g * 4 + i
                            ps = ops_[:, i * RPC:(i + 1) * RPC]
                            pslice = po[:, od * RPC:(od + 1) * RPC]
                            if h == 0:
                                # first head: plain drain (ACT, near PSUM)
                                nc.scalar.copy(pslice, ps)
                            else:
                                nc.vector.tensor_add(pslice, pslice, ps)
                            if h == HPC - 1:
                                # final: store straight from the partial
                                # buffer, alternating DMA queues (both are
                                # idle by now) to shorten the tail
                                oeng = nc.scalar if i % 2 == 0 else nc.sync
                                oeng.dma_start(
                                    out[od * 128:(od + 1) * 128, :], pslice)

            wos_cm.__exit__(None, None, None)

    nc.compile()
    return nc


def _host_prep(x, wq, wk, wv, wo):
    perm = np.concatenate([np.arange(0, HD, 2), np.arange(1, HD, 2)])
    x2 = np.ascontiguousarray(x.reshape(ROWS, D).T)        # [D, ROWS]
    xT_r = x2.reshape(DT_, 128, ROWS).astype(NP_CDT)

    wq_p = wq.reshape(D, HQ, HD)[:, :, perm].reshape(D, HQ * HD)
    wk_p = wk.reshape(D, HKV, HD)[:, :, perm].reshape(D, HKV * HD)

    # per-core weight shards in sbuf tile layout [128p, d-tile, cols]
    def tile_rows(w):  # [D, C] -> [128, DT_*C] with blocks (d, c)
        Dd, C = w.shape
        return np.ascontiguousarray(
            w.reshape(DT_, 128, C).transpose(1, 0, 2).reshape(128, DT_ * C))

    wq_cores = []
    wk_cores = []
    wv_cores = []
    for c in range(NC_):
        wqc = wq_p[:, c * HPC * HD:(c + 1) * HPC * HD]     # [D, 512]
        wq_cores.append(tile_rows(wqc).astype(NP_CDT))
        wk_cores.append(tile_rows(
            wk_p[:, c * HD:(c + 1) * HD]).astype(NP_CDT))
        wv_cores.append(tile_rows(
            wv[:, c * HD:(c + 1) * HD]).astype(NP_CDT))

    # wo lhsT tiles grouped by (od-group g, head-slot h):
    # wo_t[g*HPC+h, p, (s*4+i)*128+j] = wo[(s*HPC+h)*128+p, (g*4+i)*128+j]
    wo_t = np.ascontiguousarray(
        wo.reshape(NC_, HPC, 128, DT_ // 4, 4, 128)
        .transpose(3, 1, 2, 0, 4, 5)
        .reshape((DT_ // 4) * HPC, 128, NC_ * 4 * 128)).astype(NP_CDT)

    inv = 1.0 / (THETA ** (np.arange(0, HD, 2, dtype=np.float64) / HD))
    ang = np.arange(S, dtype=np.float64)[:, None] * inv[None, :]
    cosT = np.cos(ang).T
    sinT = np.sin(ang).T
    csa = np.concatenate([cosT, cosT], axis=0).astype(np.float32)
    csb = np.concatenate([-sinT, sinT], axis=0).astype(np.float32)

    # transposed causal mask in bf16: consumed as matmul lhsT so that
    # (m^T)^T @ I = m accumulates onto the diagonal score block on PE
    m = np.where(np.arange(HD)[None, :] > np.arange(HD)[:, None],
                 np.float32(-1e9), np.float32(0.0)).T.astype(NP_CDT)
    m = np.ascontiguousarray(m)
    ident = np.eye(128, dtype=np.float32).astype(NP_CDT)
    return (xT_r, wq_cores, wk_cores, wv_cores, wo_t, csa, csb, m, ident)


def kernel(x, wq, wk, wv, wo):
    if "nc" not in _CACHE:
        _CACHE["nc"] = _build()
    nc = _CACHE["nc"]

    xT_r, wq_c, wk_c, wv_c, wo_t, csa, csb, m, ident = _host_prep(
        np.asarray(x, np.float32), np.asarray(wq, np.float32),
        np.asarray(wk, np.float32), np.asarray(wv, np.float32),
        np.asarray(wo, np.float32))

    in_maps = []
    for c in range(NC_):
        in_maps.append({
            "xT": xT_r, "wq": wq_c[c], "wk": wk_c[c], "wv": wv_c[c],
            "wo": wo_t, "csa": csa, "csb": csb, "msk": m, "idn": ident,
        })
    res = run_bass_kernel_spmd(nc, in_maps, core_ids=list(range(NC_)))
    _CACHE["last_results"] = res

    outp = np.empty((ROWS, D), np.float32)
    for c in range(NC_):
        outp[c * RPC:(c + 1) * RPC, :] = res.results[c]["out"].T
    return outp.reshape(B, S, D)



# revision 88
# speedup vs baseline: 1.0150x; 1.0150x over previous
"""Trainium2 Bass kernel for GQA attention (B=2, S=2048, D=4096, 32 q-heads,
8 kv-heads, head_dim=128, RoPE, causal) distributed over 8 NeuronCores.

Sharding: tensor-parallel over heads for QKV+attention (core c owns q-heads
4c..4c+3 and kv-head c, all sequence positions), then per-head AllToAlls
re-shard the attention output from head-sharded to row-sharded (overlapped
with attention of later heads) so the output projection wo contracts locally;
final output rows are gathered on the host.

Device dataflow per core (identical program on all 8 cores, data differs):
  - stream x^T tiles once; accumulate K^T, V^T (own kv head) and Q^T (4 own
    heads) in PSUM; drain PSUM quickly via ScalarE copies; rotate-half RoPE
    on VectorE (weights are column-permuted on the host so RoPE pairs are
    (i, i+64) within each head); V^T -> V via PE transposes, inline per chunk.
  - causal attention per (head, batch, q-tile of 128), software-pipelined:
    scores in PSUM (PE) -> exp on ScalarE (accumulated row-sum = softmax
    denominator for free; no max subtraction needed at these magnitudes) ->
    normalize P by 1/den (VectorE) -> PE-transpose P -> P^T @ V on PE.
  - AllToAll per head (head-sharded -> row-sharded), overlapped.
  - out^T = wo^T @ attn_out^T for this core's 512 rows.
Host returns out[rows_c, :] = out_c^T.T concatenated over cores.
"""
import sys
sys.path.insert(0, "/opt/trn_rl_repo")
import math
import numpy as np

import concourse.bass as bass
import concourse.bacc as bacc
import concourse.tile as tile
import concourse.mybir as mybir
from concourse.bass_utils import run_bass_kernel_spmd
from concourse.dt import dt

B, S, D = 2, 2048, 4096
HQ, HKV, HD = 32, 8, 128
NC_ = 8                       # cores
ROWS = B * S                  # 4096
RPC = ROWS // NC_             # 512 rows per core
HPC = HQ // NC_               # 4 q-heads per core
QT = S // HD                  # 16 q-tiles per batch
DT_ = 32                      # d-tiles (D/128)
SCALE = 1.0 / math.sqrt(HD)
THETA = 10000.0

F32 = mybir.dt.float32
CDT = mybir.dt.bfloat16       # compute dtype for matmul operands
NP_CDT = dt.np(CDT)

_CACHE = {}


def _build():
    nc = bacc.Bacc("TRN2", target_bir_lowering=False, debug=False,
                   num_devices=NC_)
    AF = mybir.ActivationFunctionType

    # ---- dram I/O (names = in_maps keys) ----
    xT = nc.dram_tensor("xT", [DT_, 128, ROWS], CDT, kind="ExternalInput")
    wq = nc.dram_tensor("wq", [128, DT_ * HPC * HD], CDT, kind="ExternalInput")
    wk = nc.dram_tensor("wk", [128, DT_ * HD], CDT, kind="ExternalInput")
    wv = nc.dram_tensor("wv", [128, DT_ * HD], CDT, kind="ExternalInput")
    wo = nc.dram_tensor("wo", [(DT_ // 4) * HPC, 128, NC_ * 4 * HD], CDT,
                        kind="ExternalInput")
    csa = nc.dram_tensor("csa", [128, S], F32, kind="ExternalInput")  # cos|cos
    csb = nc.dram_tensor("csb", [128, S], F32, kind="ExternalInput")  # -sin|sin
    msk = nc.dram_tensor("msk", [128, HD], CDT, kind="ExternalInput")
    idn = nc.dram_tensor("idn", [128, 128], CDT, kind="ExternalInput")
    out = nc.dram_tensor("out", [DT_ * 128, RPC], F32, kind="ExternalOutput")

    with tile.TileContext(nc) as tc:
        with tc.tile_pool(name="const", bufs=1) as constp, \
             tc.tile_pool(name="dram", bufs=1, space="DRAM") as dram:
            # constants (cos/sin table loads are emitted after chunk 0's
            # scalar-queue weight loads; first rope use is ~45us in)
            csa_sb = constp.tile([128, S], F32)
            csb_sb = constp.tile([128, S], F32)
            msk_sb = constp.tile([128, HD], CDT)
            nc.scalar.dma_start(msk_sb[:], msk[:])
            idn_sb = constp.tile([128, 128], CDT)
            nc.scalar.dma_start(idn_sb[:], idn[:])
            # staging for head-0's a2a output slices: lives in never-released
            # SBUF so its loads run as soon as head 0's collective lands
            # (~150us before phase O needs them), with no WAR on k/q/v
            a2h0 = constp.tile([128, NC_ * RPC], CDT)

            # tiny warmup AllToAll: absorbs the one-time collective setup
            # cost (~120us on the first collective) under phase P's matmuls
            wu_in = dram.tile([NC_, 64], F32)
            wu_out = dram.tile([NC_, 64], F32)
            nc.gpsimd.collective_compute(
                "AllToAll", mybir.AluOpType.bypass,
                ins=[wu_in[:].opt()], outs=[wu_out[:].opt()],
                replica_groups=[list(range(NC_))],
            )

            # wo staging pool opened before persist (manual scoping: persist
            # closes first, wos survives into phase O) so the first wo tiles
            # can prefetch during the last attention head
            wos_cm = tc.tile_pool(name="wos", bufs=3)
            wos = wos_cm.__enter__()

            # persistent activations (pool closed before phase O so its SBUF
            # is reclaimed for the f32 partial-output buffer)
            persist_cm = tc.tile_pool(name="persist", bufs=1)
            persist = persist_cm.__enter__()
            k_sb = persist.tile([128, ROWS], CDT)           # K^T (rope'd)
            vn_sb = persist.tile([128, ROWS], CDT)          # V natural tiles
            q_sb = persist.tile([128, HPC * ROWS], CDT)     # Q^T per head
            ao_sb = persist.tile([128, HPC * ROWS], CDT)    # attn_out^T per head

            def _rope(dst, src_sb, swp_ps, pos0, n, tmp_pool):
                """dst[128, n] (SBUF CDT) = rope(src_sb[128, n] SBUF CDT),
                given swp_ps[128, n] (PSUM f32) = half-swapped src (from a PE
                matmul with the swap permutation). Positions pos0.. (one
                batch). All ops are full-tile, partition-aligned:
                  dst = src * [cos|cos] + swapped(src) * [-sin|sin]."""
                ca = csa_sb[:, pos0:pos0 + n]
                cb = csb_sb[:, pos0:pos0 + n]
                t = tmp_pool.tile([128, n], F32, tag="ropetmp")
                nc.vector.tensor_mul(t[:], src_sb[:], ca)
                u = tmp_pool.tile([128, n], F32, tag="ropetmp2")
                nc.vector.tensor_mul(u[:], swp_ps[:], cb)
                nc.vector.tensor_add(dst[:], t[:], u[:])

            # ---------------- phase P: projections ----------------
            with tc.tile_pool(name="wts", bufs=1) as wtp, \
                 tc.tile_pool(name="xs", bufs=6) as xsp, \
                 tc.tile_pool(name="ptmp", bufs=2) as ptmp, \
                 tc.tile_pool(name="drain", bufs=2) as drp, \
                 tc.tile_pool(name="pps", bufs=1, space="PSUM") as pps, \
                 tc.tile_pool(name="kpp", bufs=2, space="PSUM") as kpp, \
                 tc.tile_pool(name="miscp", bufs=1, space="PSUM") as miscp:
                # weight loads chunked (8 chunks of 4 d-tiles); chunk 0 lands
                # first so matmuls start immediately, the rest interleave
                # with the first row-chunk's xT stream
                wk_sb = wtp.tile([128, DT_ * HD], CDT)
                wv_sb = wtp.tile([128, DT_ * HD], CDT)
                wq_sb = wtp.tile([128, DT_ * HPC * HD], CDT)

                def load_wchunk(qd):
                    # chunk 0's wv/wq ride the scalar queue so the first
                    # d-iterations' weights land in parallel with wk/xt on
                    # sync; later chunks stay on sync (scalar carries the
                    # cos/sin tables then drains)
                    weng = nc.scalar if qd == 0 else nc.sync
                    c0, c1 = qd * DT_ * HD // 8, (qd + 1) * DT_ * HD // 8
                    nc.sync.dma_start(wk_sb[:, c0:c1], wk[:, c0:c1])
                    weng.dma_start(wv_sb[:, c0:c1], wv[:, c0:c1])
                    q0_, q1_ = qd * DT_ * HPC * HD // 8, \
                        (qd + 1) * DT_ * HPC * HD // 8
                    if qd == 0:
                        # d0's four head-slices first: the long wq transfer
                        # otherwise gates the first d-iteration by ~5us
                        nc.scalar.dma_start(wq_sb[:, 0:HPC * HD],
                                            wq[:, 0:HPC * HD])
                        nc.scalar.dma_start(wq_sb[:, HPC * HD:q1_],
                                            wq[:, HPC * HD:q1_])
                    else:
                        weng.dma_start(wq_sb[:, q0_:q1_], wq[:, q0_:q1_])

                # first x tile ahead of the bulk weight loads so the first
                # matmul's operands land as early as possible
                xt00 = xsp.tile([128, 512], CDT, tag="xt", name="xt_pre")
                nc.sync.dma_start(xt00[:], xT[0, :, 0:512])
                load_wchunk(0)
                nc.scalar.dma_start(csa_sb[:], csa[:])
                nc.scalar.dma_start(csb_sb[:], csb[:])

                NCH = ROWS // 512   # 8 row-chunks
                for ch in range(NCH):
                    pos0 = (ch * 512) % S
                    kp = kpp.tile([128, 512], F32, tag="kp")
                    vp = pps.tile([128, 512], F32, tag="vp")
                    qp = [pps.tile([128, 512], F32, tag=f"qp{h}",
                                   name=f"qp{h}_{ch}")
                          for h in range(HPC)]
                    for d in range(DT_):
                        if ch == 0 and d == 0:
                            xt = xt00
                        else:
                            xt = xsp.tile([128, 512], CDT, tag="xt")
                            nc.sync.dma_start(
                                xt[:], xT[d, :, ch * 512:(ch + 1) * 512])
                        if ch == 0 and d % 4 == 0 and d // 4 + 1 < 8:
                            load_wchunk(d // 4 + 1)
                        st, sp = (d == 0), (d == DT_ - 1)
                        # order matches drain completion: kp double-buffered
                        # (never waits), qp2/qp3 drained on DVE, vp/qp0/qp1
                        # on ACT
                        nc.tensor.matmul(kp[:], wk_sb[:, d * HD:(d + 1) * HD],
                                         xt[:], start=st, stop=sp)
                        for h in (2, 3):
                            w0 = (d * HPC + h) * HD
                            nc.tensor.matmul(qp[h][:],
                                             wq_sb[:, w0:w0 + HD],
                                             xt[:], start=st, stop=sp)
                        nc.tensor.matmul(vp[:], wv_sb[:, d * HD:(d + 1) * HD],
                                         xt[:], start=st, stop=sp)
                        for h in (0, 1):
                            w0 = (d * HPC + h) * HD
                            nc.tensor.matmul(qp[h][:],
                                             wq_sb[:, w0:w0 + HD],
                                             xt[:], start=st, stop=sp)
                    sl = slice(ch * 512, (ch + 1) * 512)
                    # drain PSUM fast (cast to bf16): vp/qp0/qp1 on ACT,
                    # qp2/qp3 on DVE, kf last on ACT (kp is double-buffered
                    # so its drain only gates the rope, not the next chunk)
                    qf = [drp.tile([128, 512], CDT, tag=f"qf{h}",
                                   name=f"qf{h}_{ch}")
                          for h in range(HPC)]
                    vf = drp.tile([128, 512], CDT, tag="vf")
                    nc.scalar.copy(vf[:], vp[:])   # V^T chunk (bf16)
                    nc.vector.tensor_copy(qf[2][:], qp[2][:])
                    nc.vector.tensor_copy(qf[3][:], qp[3][:])
                    nc.scalar.copy(qf[0][:], qp[0][:])
                    nc.scalar.copy(qf[1][:], qp[1][:])
                    kf = drp.tile([128, 512], CDT, tag="kf")
                    nc.scalar.copy(kf[:], kp[:])
                    # rotate-half via partition-offset DMA copies on the SP
                    # HWDGE ring instead of PE permutation matmuls: removes
                    # 5 N=512 matmuls/chunk from the 99%-busy PE stream.
                    # (Issuing from the ACT queue instead stalls ScalarE's
                    # PSUM drains; the SP queue only carries DMAs here and
                    # the xt prefetch depth absorbs the extra transfers.)
                    def _swap(src, nm):
                        sw = drp.tile([128, 512], CDT, tag="swp",
                                      name=f"swp_{ch}_{nm}")
                        nc.sync.dma_start(sw[0:64, :], src[64:128, :])
                        nc.sync.dma_start(sw[64:128, :], src[0:64, :])
                        return sw
                    _rope(k_sb[:, sl], kf[:], _swap(kf[:], "k"), pos0, 512,
                          ptmp)
                    for h in range(HPC):
                        _rope(q_sb[:, h * ROWS + ch * 512:
                                   h * ROWS + (ch + 1) * 512],
                              qf[h][:], _swap(qf[h][:], f"q{h}"), pos0, 512,
                              ptmp)
                    # V^T -> V natural, inline (PE transposes + DVE drain)
                    vt = miscp.tile([128, 512], CDT, tag="misc",
                                    name=f"vt_{ch}")
                    for t in range(4):
                        nc.tensor.transpose(vt[:, t * 128:(t + 1) * 128],
                                            vf[:, t * 128:(t + 1) * 128],
                                            idn_sb[:])
                    nc.vector.tensor_copy(vn_sb[:, sl], vt[:])

            # ---------------- phase A: attention (+ per-head A2A) --------
            # separate DRAM tiles per head: DRAM dep-tracking is tensor-
            # granular, so a shared buffer makes head h+1's stores falsely
            # wait on collective h's reads (serializing the A2A pipeline)
            a2a_in = [dram.tile([NC_, 128, RPC], CDT, name=f"a2ai{h}")
                      for h in range(HPC)]
            a2a_out = [dram.tile([NC_, 128, RPC], CDT, name=f"a2ao{h}")
                       for h in range(HPC)]

            with tc.tile_pool(name="att", bufs=3) as att, \
                 tc.tile_pool(name="attd", bufs=6) as attd, \
                 tc.tile_pool(name="sps", bufs=2, space="PSUM") as sps, \
                 tc.tile_pool(name="tps2", bufs=2, space="PSUM") as tps2, \
                 tc.tile_pool(name="ops", bufs=2, space="PSUM") as ops:

                def stage1(h, b, j):
                    """QK -> exp -> den/normalize -> xbar-transposed P^T."""
                    klen = HD * (j + 1)
                    q0 = h * ROWS + b * S + j * HD
                    qt_ap = q_sb[:, q0:q0 + HD]
                    nkc = (klen + 1023) // 1024
                    p_t = att.tile([128, 2048], CDT, tag="p",
                                   name=f"p_{h}_{b}_{j}")
                    den = attd.tile([128, 4], F32, tag="den",
                                    name=f"den_{h}_{b}_{j}")
                    for kc in range(nkc):
                        k0 = kc * 1024
                        kl = min(1024, klen - k0)
                        sp_ = sps.tile([128, 1024], F32, tag="sp",
                                       name=f"sp_{h}_{b}_{j}_{kc}")
                        for nn in range(0, kl, 512):
                            nw = min(512, kl - nn)
                            if k0 + nn + nw == klen:
                                # final block: causal mask accumulated on PE
                                # via mask^T @ I (PE has slack once the P
                                # transposes move to the DMA crossbar)
                                if nw > HD:
                                    nc.tensor.matmul(
                                        sp_[:, nn:nn + nw - HD], qt_ap,
                                        k_sb[:, b * S + k0 + nn:
                                             b * S + k0 + nn + nw - HD],
                                        start=True, stop=True)
                                d0 = nn + nw - HD
                                nc.tensor.matmul(
                                    sp_[:, d0:d0 + HD], qt_ap,
                                    k_sb[:, b * S + klen - HD:b * S + klen],
                                    start=True, stop=False)
                                nc.tensor.matmul(
                                    sp_[:, d0:d0 + HD], msk_sb[:], idn_sb[:],
                                    start=False, stop=True)
                            else:
                                nc.tensor.matmul(
                                    sp_[:, nn:nn + nw], qt_ap,
                                    k_sb[:, b * S + k0 + nn:
                                         b * S + k0 + nn + nw],
                                    start=True, stop=True)
                        nc.scalar.activation(
                            p_t[:, k0:k0 + kl], sp_[:, 0:kl],
                            AF.Exp, scale=SCALE,
                            accum_out=den[:, kc:kc + 1])
                    for kc in range(1, nkc):
                        nc.vector.tensor_add(den[:, 0:1], den[:, 0:1],
                                             den[:, kc:kc + 1])
                    rden = attd.tile([128, 1], F32, tag="rden",
                                     name=f"rden_{h}_{b}_{j}")
                    nc.vector.reciprocal(rden[:], den[:, 0:1])
                    nc.vector.tensor_scalar_mul(p_t[:, 0:klen],
                                                p_t[:, 0:klen], rden[:])
                    return p_t

                def stage2a(h, b, j, p_t):
                    """P transpose (PE) + PSUM->SBUF copies (DVE)."""
                    pt_t = att.tile([128, 2048], CDT, tag="pt",
                                    name=f"pt_{h}_{b}_{j}")
                    for g in range(0, j + 1, 4):
                        gw = min(4, j + 1 - g)
                        tp = tps2.tile([128, 512], CDT, tag="tp",
                                       name=f"tp_{h}_{b}_{j}_{g}")
                        for t in range(gw):
                            c0 = (g + t) * HD
                            nc.tensor.transpose(tp[:, t * HD:(t + 1) * HD],
                                                p_t[:, c0:c0 + HD], idn_sb[:])
                        nc.vector.tensor_copy(pt_t[:, g * HD:(g + gw) * HD],
                                              tp[:, 0:gw * HD])
                    return pt_t

                def stage2b(h, b, j, pt_t, otg):
                    """PV accumulate -> drain OT group. Runs one tile behind
                    stage2a so the PV never waits on the fresh P^T copy."""
                    jj = j % 4
                    for kt in range(j + 1):
                        nc.tensor.matmul(
                            otg[:, jj * HD:(jj + 1) * HD],
                            vn_sb[:, (b * QT + kt) * HD:(b * QT + kt + 1) * HD],
                            pt_t[:, kt * HD:(kt + 1) * HD],
                            start=(kt == 0), stop=(kt == j))
                    if jj == 0:
                        # group complete (descending j): drain 4 OTs at once
                        g0 = h * ROWS + b * S + j * HD
                        nc.vector.tensor_copy(
                            ao_sb[:, g0:g0 + 4 * HD], otg[:])

                wo_pre = {}
                for h in range(HPC):
                    if h == HPC - 1:
                        # prefetch the first two phase-O weight tiles while
                        # the sync queue is otherwise idle
                        for g in (0, 1):
                            wt = wos.tile([128, NC_ * 4 * HD], CDT, tag="wo",
                                          name=f"wo_{g}_0")
                            nc.sync.dma_start(wt[:], wo[g * HPC, :, :])
                            wo_pre[(g, 0)] = wt
                    pend_a = []
                    pend_b = []
                    otg = [None]

                    def run_b(args):
                        hh, bb, jj_, pt_t = args
                        if jj_ % 4 == 3:
                            otg[0] = ops.tile([128, 512], F32, tag="ot",
                                              name=f"ot_{hh}_{bb}_{jj_}")
                        stage2b(hh, bb, jj_, pt_t, otg[0])

                    # q-tiles in descending j: the big tiles lead (hiding the
                    # exp/normalize latency at pipeline fill), the tiny ones
                    # flush the tail just before this head's AllToAll
                    for b in range(B):
                        for j in range(QT - 1, -1, -1):
                            p_t = stage1(h, b, j)
                            pend_a.append((h, b, j, p_t))
                            if len(pend_a) > 2:
                                ha, ba, ja, pa = pend_a.pop(0)
                                pend_b.append((ha, ba, ja,
                                               stage2a(ha, ba, ja, pa)))
                            if len(pend_b) > 1:
                                run_b(pend_b.pop(0))
                    for ha, ba, ja, pa in pend_a:
                        pend_b.append((ha, ba, ja, stage2a(ha, ba, ja, pa)))
                    for args in pend_b:
                        run_b(args)
                    # per-head AllToAll, overlaps later heads' attention;
                    # stores on sync+gpsimd only -- NEVER the scalar queue
                    # (a blocked store there stalls the in-order exp stream)
                    for r in range(NC_):
                        seng = nc.sync if r % 2 == 0 else nc.gpsimd
                        seng.dma_start(
                            a2a_in[h][r, :, :],
                            ao_sb[:, h * ROWS + r * RPC:
                                  h * ROWS + (r + 1) * RPC])
                    nc.gpsimd.collective_compute(
                        "AllToAll", mybir.AluOpType.bypass,
                        ins=[a2a_in[h][:].opt()], outs=[a2a_out[h][:].opt()],
                        replica_groups=[list(range(NC_))],
                    )

            persist_cm.__exit__(None, None, None)

            # ---------------- phase O: output projection ----------------
            # h-MAJOR passes with SBUF-accumulated partials: head h's A2A is
            # only needed ~67us*h into phase O, so even ~100us of inter-core
            # arrival skew on the collectives stays off the critical path.
            # PSUM accumulates over sources s within one (g,h) pass; the
            # cross-h accumulation runs on the otherwise-idle DVE in SBUF.
            NG = DT_ // 4   # 8 od-groups of 4 PSUM banks, double-buffered
            with tc.tile_pool(name="osb", bufs=1) as osb, \
                 tc.tile_pool(name="opp", bufs=2, space="PSUM") as opp:
                # NOTE: each tile allocation must immediately precede its
                # first writer — the released-persist-zone dependency is
                # attached to the next emitted instruction, so reordering
                # allocations against the load/sweep emission corrupts data
                # (measured: NaN / 1e-1 errors with po allocated early).
                ao2 = osb.tile([128, DT_ * RPC], CDT)
                # ALL a2a-output loads stay on the gpsimd queue: they wait on
                # collective completion, and the scheduler may hoist them —
                # on sync/scalar such a blocked DMA stalls the exp stream and
                # the whole machine (measured 4-12us global quiesce)
                for h in range(HPC):
                    for s_ in range(NC_):
                        ct = s_ * HPC + h
                        if h == 0:
                            # head 0 -> the early-loadable constp staging
                            nc.gpsimd.dma_start(
                                a2h0[:, s_ * RPC:(s_ + 1) * RPC],
                                a2a_out[h][s_, :, :])
                        else:
                            nc.gpsimd.dma_start(
                                ao2[:, ct * RPC:(ct + 1) * RPC],
                                a2a_out[h][s_, :, :])
                po = osb.tile([128, DT_ * RPC], F32)   # partial out, 32 od
                for h in range(HPC):
                    for g in range(NG):
                        ops_ = opp.tile([128, 4 * RPC], F32, tag="op",
                                        name=f"op_{g}_{h}")
                        if (g, h) in wo_pre:
                            wo_sb = wo_pre.pop((g, h))
                        else:
                            wo_sb = wos.tile([128, NC_ * 4 * HD], CDT,
                                             tag="wo", name=f"wo_{g}_{h}")
                            # alternate queues: halves load latency and keeps
                            # the first passes off the a2a-store backlog
                            weng = nc.sync if (h * NG + g) % 2 == 0 \
                                else nc.scalar
                            weng.dma_start(wo_sb[:], wo[g * HPC + h, :, :])
                        for s_ in range(NC_):
                            ct = s_ * HPC + h
                            rhs = (a2h0[:, s_ * RPC:(s_ + 1) * RPC]
                                   if h == 0 else
                                   ao2[:, ct * RPC:(ct + 1) * RPC])
                            for i in range(4):
                                w0 = (s_ * 4 + i) * HD
                                nc.tensor.matmul(
                                    ops_[:, i * RPC:(i + 1) * RPC],
                                    wo_sb[:, w0:w0 + HD], rhs,
                                    start=(s_ == 0), stop=(s_ == NC_ - 1))
                        for i in range(4):
                            od = g * 4 + i
                            ps = ops_[:, i * RPC:(i + 1) * RPC]
                            pslice = po[:, od * RPC:(od + 1) * RPC]
                            if h == 0:
                                # first head: plain drain (ACT, near PSUM)
                                nc.scalar.copy(pslice, ps)
                            else:
                                nc.vector.tensor_add(pslice, pslice, ps)
                            if h == HPC - 1:
                                # final: store straight from the partial
                                # buffer, alternating DMA queues (both are
                                # idle by now) to shorten the tail
                                oeng = nc.scalar if i % 2 == 0 else nc.sync
                                oeng.dma_start(
                                    out[od * 128:(od + 1) * 128, :], pslice)

            wos_cm.__exit__(None, None, None)

    nc.compile()
    return nc


def _host_prep(x, wq, wk, wv, wo):
    perm = np.concatenate([np.arange(0, HD, 2), np.arange(1, HD, 2)])
    x2 = np.ascontiguousarray(x.reshape(ROWS, D).T)        # [D, ROWS]
    xT_r = x2.reshape(DT_, 128, ROWS).astype(NP_CDT)

    wq_p = wq.reshape(D, HQ, HD)[:, :, perm].reshape(D, HQ * HD)
    wk_p = wk.reshape(D, HKV, HD)[:, :, perm].reshape(D, HKV * HD)

    # per-core weight shards in sbuf tile layout [128p, d-tile, cols]
    def tile_rows(w):  # [D, C] -> [128, DT_*C] with blocks (d, c)
        Dd, C = w.shape
        return np.ascontiguousarray(
            w.reshape(DT_, 128, C).transpose(1, 0, 2).reshape(128, DT_ * C))

    wq_cores = []
    wk_cores = []
    wv_cores = []
    for c in range(NC_):
        wqc = wq_p[:, c * HPC * HD:(c + 1) * HPC * HD]     # [D, 512]
        wq_cores.append(tile_rows(wqc).astype(NP_CDT))
        wk_cores.append(tile_rows(
            wk_p[:, c * HD:(c + 1) * HD]).astype(NP_CDT))
        wv_cores.append(tile_rows(
            wv[:, c * HD:(c + 1) * HD]).astype(NP_CDT))

    # wo lhsT tiles grouped by (od-group g, head-slot h):
    # wo_t[g*HPC+h, p, (s*4+i)*128+j] = wo[(s*HPC+h)*128+p, (g*4+i)*128+j]
    wo_t = np.ascontiguousarray(
        wo.reshape(NC_, HPC, 128, DT_ // 4, 4, 128)
        .transpose(3, 1, 2, 0, 4, 5)
        .reshape((DT_ // 4) * HPC, 128, NC_ * 4 * 128)).astype(NP_CDT)

    inv = 1.0 / (THETA ** (np.arange(0, HD, 2, dtype=np.float64) / HD))
    ang = np.arange(S, dtype=np.float64)[:, None] * inv[None, :]
    cosT = np.cos(ang).T
    sinT = np.sin(ang).T
    csa = np.concatenate([cosT, cosT], axis=0).astype(np.float32)
    csb = np.concatenate([-sinT, sinT], axis=0).astype(np.float32)

    # transposed causal mask in bf16: consumed as matmul lhsT so that
    # (m^T)^T @ I = m accumulates onto the diagonal score block on PE
    m = np.where(np.arange(HD)[None, :] > np.arange(HD)[:, None],
                 np.float32(-1e9), np.float32(0.0)).T.astype(NP_CDT)
    m = np.ascontiguousarray(m)
    ident = np.eye(128, dtype=np.float32).astype(NP_CDT)
    return (xT_r, wq_cores, wk_cores, wv_cores, wo_t, csa, csb, m, ident)


def kernel(x, wq, wk, wv, wo):
    if "nc" not in _CACHE:
        _CACHE["nc"] = _build()
    nc = _CACHE["nc"]

    xT_r, wq_c, wk_c, wv_c, wo_t, csa, csb, m, ident = _host_prep(
        np.asarray(x, np.float32), np.asarray(wq, np.float32),
        np.asarray(wk, np.float32), np.asarray(wv, np.float32),
        np.asarray(wo, np.float32))

    in_maps = []
    for c in range(NC_):
        in_maps.append({
            "xT": xT_r, "wq": wq_c[c], "wk": wk_c[c], "wv": wv_c[c],
            "wo": wo_t, "csa": csa, "csb": csb, "msk": m, "idn": ident,
        })
    res = run_bass_kernel_spmd(nc, in_maps, core_ids=list(range(NC_)))
    _CACHE["last_results"] = res

    outp = np.empty((ROWS, D), np.float32)
    for c in range(NC_):
        outp[c * RPC:(c + 1) * RPC, :] = res.results[c]["out"].T
    return outp.reshape(B, S, D)



# revision 90
# speedup vs baseline: 1.0190x; 1.0039x over previous
"""Trainium2 Bass kernel for GQA attention (B=2, S=2048, D=4096, 32 q-heads,
8 kv-heads, head_dim=128, RoPE, causal) distributed over 8 NeuronCores.

Sharding: tensor-parallel over heads for QKV+attention (core c owns q-heads
4c..4c+3 and kv-head c, all sequence positions), then per-head AllToAlls
re-shard the attention output from head-sharded to row-sharded (overlapped
with attention of later heads) so the output projection wo contracts locally;
final output rows are gathered on the host.

Device dataflow per core (identical program on all 8 cores, data differs):
  - stream x^T tiles once; accumulate K^T, V^T (own kv head) and Q^T (4 own
    heads) in PSUM; drain PSUM quickly via ScalarE copies; rotate-half RoPE
    on VectorE (weights are column-permuted on the host so RoPE pairs are
    (i, i+64) within each head); V^T -> V via PE transposes, inline per chunk.
  - causal attention per (head, batch, q-tile of 128), software-pipelined:
    scores in PSUM (PE) -> exp on ScalarE (accumulated row-sum = softmax
    denominator for free; no max subtraction needed at these magnitudes) ->
    normalize P by 1/den (VectorE) -> PE-transpose P -> P^T @ V on PE.
  - AllToAll per head (head-sharded -> row-sharded), overlapped.
  - out^T = wo^T @ attn_out^T for this core's 512 rows.
Host returns out[rows_c, :] = out_c^T.T concatenated over cores.
"""
import sys
sys.path.insert(0, "/opt/trn_rl_repo")
import math
import numpy as np

import concourse.bass as bass
import concourse.bacc as bacc
import concourse.tile as tile
import concourse.mybir as mybir
from concourse.bass_utils import run_bass_kernel_spmd
from concourse.dt import dt

B, S, D = 2, 2048, 4096
HQ, HKV, HD = 32, 8, 128
NC_ = 8                       # cores
ROWS = B * S                  # 4096
RPC = ROWS // NC_             # 512 rows per core
HPC = HQ // NC_               # 4 q-heads per core
QT = S // HD                  # 16 q-tiles per batch
DT_ = 32                      # d-tiles (D/128)
SCALE = 1.0 / math.sqrt(HD)
THETA = 10000.0

F32 = mybir.dt.float32
CDT = mybir.dt.bfloat16       # compute dtype for matmul operands
NP_CDT = dt.np(CDT)

_CACHE = {}


def _build():
    nc = bacc.Bacc("TRN2", target_bir_lowering=False, debug=False,
                   num_devices=NC_)
    AF = mybir.ActivationFunctionType

    # ---- dram I/O (names = in_maps keys) ----
    xT = nc.dram_tensor("xT", [DT_, 128, ROWS], CDT, kind="ExternalInput")
    wq = nc.dram_tensor("wq", [128, DT_ * HPC * HD], CDT, kind="ExternalInput")
    wk = nc.dram_tensor("wk", [128, DT_ * HD], CDT, kind="ExternalInput")
    wv = nc.dram_tensor("wv", [128, DT_ * HD], CDT, kind="ExternalInput")
    wo = nc.dram_tensor("wo", [(DT_ // 4) * HPC, 128, NC_ * 4 * HD], CDT,
                        kind="ExternalInput")
    csa = nc.dram_tensor("csa", [128, S], F32, kind="ExternalInput")  # cos|cos
    csb = nc.dram_tensor("csb", [128, S], F32, kind="ExternalInput")  # -sin|sin
    msk = nc.dram_tensor("msk", [128, HD], CDT, kind="ExternalInput")
    idn = nc.dram_tensor("idn", [128, 128], CDT, kind="ExternalInput")
    out = nc.dram_tensor("out", [DT_ * 128, RPC], F32, kind="ExternalOutput")

    with tile.TileContext(nc) as tc:
        with tc.tile_pool(name="const", bufs=1) as constp, \
             tc.tile_pool(name="dram", bufs=1, space="DRAM") as dram:
            # constants (cos/sin table loads are emitted after chunk 0's
            # scalar-queue weight loads; first rope use is ~45us in)
            csa_sb = constp.tile([128, S], F32)
            csb_sb = constp.tile([128, S], F32)
            msk_sb = constp.tile([128, HD], CDT)
            nc.scalar.dma_start(msk_sb[:], msk[:])
            idn_sb = constp.tile([128, 128], CDT)
            nc.scalar.dma_start(idn_sb[:], idn[:])
            # staging for head-0's a2a output slices: lives in never-released
            # SBUF so its loads run as soon as head 0's collective lands
            # (~150us before phase O needs them), with no WAR on k/q/v
            a2h0 = constp.tile([128, NC_ * RPC], CDT)

            # tiny warmup AllToAll: absorbs the one-time collective setup
            # cost (~120us on the first collective) under phase P's matmuls
            wu_in = dram.tile([NC_, 64], F32)
            wu_out = dram.tile([NC_, 64], F32)
            nc.gpsimd.collective_compute(
                "AllToAll", mybir.AluOpType.bypass,
                ins=[wu_in[:].opt()], outs=[wu_out[:].opt()],
                replica_groups=[list(range(NC_))],
            )

            # wo staging pool opened before persist (manual scoping: persist
            # closes first, wos survives into phase O) so the first wo tiles
            # can prefetch during the last attention head
            wos_cm = tc.tile_pool(name="wos", bufs=3)
            wos = wos_cm.__enter__()

            # persistent activations (pool closed before phase O so its SBUF
            # is reclaimed for the f32 partial-output buffer)
            persist_cm = tc.tile_pool(name="persist", bufs=1)
            persist = persist_cm.__enter__()
            k_sb = persist.tile([128, ROWS], CDT)           # K^T (rope'd)
            vn_sb = persist.tile([128, ROWS], CDT)          # V natural tiles
            q_sb = persist.tile([128, HPC * ROWS], CDT)     # Q^T per head
            ao_sb = persist.tile([128, HPC * ROWS], CDT)    # attn_out^T per head

            def _rope(dst, src_sb, swp_ps, pos0, n, tmp_pool):
                """dst[128, n] (SBUF CDT) = rope(src_sb[128, n] SBUF CDT),
                given swp_ps[128, n] (PSUM f32) = half-swapped src (from a PE
                matmul with the swap permutation). Positions pos0.. (one
                batch). All ops are full-tile, partition-aligned:
                  dst = src * [cos|cos] + swapped(src) * [-sin|sin]."""
                ca = csa_sb[:, pos0:pos0 + n]
                cb = csb_sb[:, pos0:pos0 + n]
                t = tmp_pool.tile([128, n], F32, tag="ropetmp")
                nc.vector.tensor_mul(t[:], src_sb[:], ca)
                u = tmp_pool.tile([128, n], F32, tag="ropetmp2")
                nc.vector.tensor_mul(u[:], swp_ps[:], cb)
                nc.vector.tensor_add(dst[:], t[:], u[:])

            # ---------------- phase P: projections ----------------
            with tc.tile_pool(name="wts", bufs=1) as wtp, \
                 tc.tile_pool(name="xs", bufs=6) as xsp, \
                 tc.tile_pool(name="ptmp", bufs=2) as ptmp, \
                 tc.tile_pool(name="drain", bufs=2) as drp, \
                 tc.tile_pool(name="pps", bufs=1, space="PSUM") as pps, \
                 tc.tile_pool(name="kpp", bufs=2, space="PSUM") as kpp, \
                 tc.tile_pool(name="miscp", bufs=1, space="PSUM") as miscp:
                # weight loads chunked (8 chunks of 4 d-tiles); chunk 0 lands
                # first so matmuls start immediately, the rest interleave
                # with the first row-chunk's xT stream
                wk_sb = wtp.tile([128, DT_ * HD], CDT)
                wv_sb = wtp.tile([128, DT_ * HD], CDT)
                wq_sb = wtp.tile([128, DT_ * HPC * HD], CDT)

                def load_wchunk(qd):
                    # chunk 0's wv/wq ride the scalar queue so the first
                    # d-iterations' weights land in parallel with wk/xt on
                    # sync; later chunks stay on sync (scalar carries the
                    # cos/sin tables then drains)
                    weng = nc.scalar if qd == 0 else nc.sync
                    c0, c1 = qd * DT_ * HD // 8, (qd + 1) * DT_ * HD // 8
                    nc.sync.dma_start(wk_sb[:, c0:c1], wk[:, c0:c1])
                    weng.dma_start(wv_sb[:, c0:c1], wv[:, c0:c1])
                    q0_, q1_ = qd * DT_ * HPC * HD // 8, \
                        (qd + 1) * DT_ * HPC * HD // 8
                    if qd == 0:
                        # d0's four head-slices first: the long wq transfer
                        # otherwise gates the first d-iteration by ~5us
                        nc.scalar.dma_start(wq_sb[:, 0:HPC * HD],
                                            wq[:, 0:HPC * HD])
                        nc.scalar.dma_start(wq_sb[:, HPC * HD:q1_],
                                            wq[:, HPC * HD:q1_])
                    else:
                        weng.dma_start(wq_sb[:, q0_:q1_], wq[:, q0_:q1_])

                # first x tile ahead of the bulk weight loads so the first
                # matmul's operands land as early as possible
                xt00 = xsp.tile([128, 512], CDT, tag="xt", name="xt_pre")
                nc.sync.dma_start(xt00[:], xT[0, :, 0:512])
                load_wchunk(0)
                nc.scalar.dma_start(csa_sb[:], csa[:])
                nc.scalar.dma_start(csb_sb[:], csb[:])

                NCH = ROWS // 512   # 8 row-chunks
                for ch in range(NCH):
                    pos0 = (ch * 512) % S
                    kp = kpp.tile([128, 512], F32, tag="kp")
                    vp = pps.tile([128, 512], F32, tag="vp")
                    qp = [pps.tile([128, 512], F32, tag=f"qp{h}",
                                   name=f"qp{h}_{ch}")
                          for h in range(HPC)]
                    for d in range(DT_):
                        if ch == 0 and d == 0:
                            xt = xt00
                        else:
                            xt = xsp.tile([128, 512], CDT, tag="xt")
                            nc.sync.dma_start(
                                xt[:], xT[d, :, ch * 512:(ch + 1) * 512])
                        if ch == 0 and d % 4 == 0 and d // 4 + 1 < 8:
                            load_wchunk(d // 4 + 1)
                        st, sp = (d == 0), (d == DT_ - 1)
                        # order matches drain completion: kp double-buffered
                        # (never waits), qp2/qp3 drained on DVE, vp/qp0/qp1
                        # on ACT
                        nc.tensor.matmul(kp[:], wk_sb[:, d * HD:(d + 1) * HD],
                                         xt[:], start=st, stop=sp)
                        for h in (2, 3):
                            w0 = (d * HPC + h) * HD
                            nc.tensor.matmul(qp[h][:],
                                             wq_sb[:, w0:w0 + HD],
                                             xt[:], start=st, stop=sp)
                        nc.tensor.matmul(vp[:], wv_sb[:, d * HD:(d + 1) * HD],
                                         xt[:], start=st, stop=sp)
                        for h in (0, 1):
                            w0 = (d * HPC + h) * HD
                            nc.tensor.matmul(qp[h][:],
                                             wq_sb[:, w0:w0 + HD],
                                             xt[:], start=st, stop=sp)
                    sl = slice(ch * 512, (ch + 1) * 512)
                    # drain PSUM fast (cast to bf16): vp/qp0/qp1 on ACT,
                    # qp2/qp3 on DVE, kf last on ACT (kp is double-buffered
                    # so its drain only gates the rope, not the next chunk)
                    qf = [drp.tile([128, 512], CDT, tag=f"qf{h}",
                                   name=f"qf{h}_{ch}")
                          for h in range(HPC)]
                    vf = drp.tile([128, 512], CDT, tag="vf")
                    nc.scalar.copy(vf[:], vp[:])   # V^T chunk (bf16)
                    nc.vector.tensor_copy(qf[2][:], qp[2][:])
                    nc.vector.tensor_copy(qf[3][:], qp[3][:])
                    nc.scalar.copy(qf[0][:], qp[0][:])
                    nc.scalar.copy(qf[1][:], qp[1][:])
                    kf = drp.tile([128, 512], CDT, tag="kf")
                    nc.scalar.copy(kf[:], kp[:])
                    # rotate-half via partition-offset DMA copies on the SP
                    # HWDGE ring instead of PE permutation matmuls: removes
                    # 5 N=512 matmuls/chunk from the 99%-busy PE stream.
                    # (Issuing from the ACT queue instead stalls ScalarE's
                    # PSUM drains; the SP queue only carries DMAs here and
                    # the xt prefetch depth absorbs the extra transfers.)
                    def _swap(src, nm):
                        sw = drp.tile([128, 512], CDT, tag="swp",
                                      name=f"swp_{ch}_{nm}")
                        nc.sync.dma_start(sw[0:64, :], src[64:128, :])
                        nc.sync.dma_start(sw[64:128, :], src[0:64, :])
                        return sw
                    _rope(k_sb[:, sl], kf[:], _swap(kf[:], "k"), pos0, 512,
                          ptmp)
                    for h in range(HPC):
                        _rope(q_sb[:, h * ROWS + ch * 512:
                                   h * ROWS + (ch + 1) * 512],
                              qf[h][:], _swap(qf[h][:], f"q{h}"), pos0, 512,
                              ptmp)
                    # V^T -> V natural, inline (PE transposes + DVE drain)
                    vt = miscp.tile([128, 512], CDT, tag="misc",
                                    name=f"vt_{ch}")
                    for t in range(4):
                        nc.tensor.transpose(vt[:, t * 128:(t + 1) * 128],
                                            vf[:, t * 128:(t + 1) * 128],
                                            idn_sb[:])
                    nc.vector.tensor_copy(vn_sb[:, sl], vt[:])

            # ---------------- phase A: attention (+ per-head A2A) --------
            # separate DRAM tiles per head: DRAM dep-tracking is tensor-
            # granular, so a shared buffer makes head h+1's stores falsely
            # wait on collective h's reads (serializing the A2A pipeline)
            a2a_in = [dram.tile([NC_, 128, RPC], CDT, name=f"a2ai{h}")
                      for h in range(HPC)]
            a2a_out = [dram.tile([NC_, 128, RPC], CDT, name=f"a2ao{h}")
                       for h in range(HPC)]

            with tc.tile_pool(name="att", bufs=4) as att, \
                 tc.tile_pool(name="attd", bufs=6) as attd, \
                 tc.tile_pool(name="sps", bufs=2, space="PSUM") as sps, \
                 tc.tile_pool(name="tps2", bufs=2, space="PSUM") as tps2, \
                 tc.tile_pool(name="ops", bufs=2, space="PSUM") as ops:

                def stage1(h, b, j):
                    """QK -> exp -> den/normalize -> xbar-transposed P^T."""
                    klen = HD * (j + 1)
                    q0 = h * ROWS + b * S + j * HD
                    qt_ap = q_sb[:, q0:q0 + HD]
                    nkc = (klen + 1023) // 1024
                    p_t = att.tile([128, 2048], CDT, tag="p",
                                   name=f"p_{h}_{b}_{j}")
                    den = attd.tile([128, 4], F32, tag="den",
                                    name=f"den_{h}_{b}_{j}")
                    for kc in range(nkc):
                        k0 = kc * 1024
                        kl = min(1024, klen - k0)
                        sp_ = sps.tile([128, 1024], F32, tag="sp",
                                       name=f"sp_{h}_{b}_{j}_{kc}")
                        for nn in range(0, kl, 512):
                            nw = min(512, kl - nn)
                            if k0 + nn + nw == klen:
                                # final block: causal mask accumulated on PE
                                # via mask^T @ I (PE has slack once the P
                                # transposes move to the DMA crossbar)
                                if nw > HD:
                                    nc.tensor.matmul(
                                        sp_[:, nn:nn + nw - HD], qt_ap,
                                        k_sb[:, b * S + k0 + nn:
                                             b * S + k0 + nn + nw - HD],
                                        start=True, stop=True)
                                d0 = nn + nw - HD
                                nc.tensor.matmul(
                                    sp_[:, d0:d0 + HD], qt_ap,
                                    k_sb[:, b * S + klen - HD:b * S + klen],
                                    start=True, stop=False)
                                nc.tensor.matmul(
                                    sp_[:, d0:d0 + HD], msk_sb[:], idn_sb[:],
                                    start=False, stop=True)
                            else:
                                nc.tensor.matmul(
                                    sp_[:, nn:nn + nw], qt_ap,
                                    k_sb[:, b * S + k0 + nn:
                                         b * S + k0 + nn + nw],
                                    start=True, stop=True)
                        nc.scalar.activation(
                            p_t[:, k0:k0 + kl], sp_[:, 0:kl],
                            AF.Exp, scale=SCALE,
                            accum_out=den[:, kc:kc + 1])
                    for kc in range(1, nkc):
                        nc.vector.tensor_add(den[:, 0:1], den[:, 0:1],
                                             den[:, kc:kc + 1])
                    rden = attd.tile([128, 1], F32, tag="rden",
                                     name=f"rden_{h}_{b}_{j}")
                    nc.vector.reciprocal(rden[:], den[:, 0:1])
                    nc.vector.tensor_scalar_mul(p_t[:, 0:klen],
                                                p_t[:, 0:klen], rden[:])
                    return p_t

                def stage2a(h, b, j, p_t):
                    """P transpose (PE) + PSUM->SBUF copies (DVE)."""
                    pt_t = att.tile([128, 2048], CDT, tag="pt",
                                    name=f"pt_{h}_{b}_{j}")
                    for g in range(0, j + 1, 4):
                        gw = min(4, j + 1 - g)
                        tp = tps2.tile([128, 512], CDT, tag="tp",
                                       name=f"tp_{h}_{b}_{j}_{g}")
                        for t in range(gw):
                            c0 = (g + t) * HD
                            nc.tensor.transpose(tp[:, t * HD:(t + 1) * HD],
                                                p_t[:, c0:c0 + HD], idn_sb[:])
                        nc.vector.tensor_copy(pt_t[:, g * HD:(g + gw) * HD],
                                              tp[:, 0:gw * HD])
                    return pt_t

                def stage2b(h, b, j, pt_t, otg):
                    """PV accumulate -> drain OT group. Runs one tile behind
                    stage2a so the PV never waits on the fresh P^T copy."""
                    jj = j % 4
                    for kt in range(j + 1):
                        nc.tensor.matmul(
                            otg[:, jj * HD:(jj + 1) * HD],
                            vn_sb[:, (b * QT + kt) * HD:(b * QT + kt + 1) * HD],
                            pt_t[:, kt * HD:(kt + 1) * HD],
                            start=(kt == 0), stop=(kt == j))
                    if jj == 0:
                        # group complete (descending j): drain 4 OTs at once
                        g0 = h * ROWS + b * S + j * HD
                        nc.vector.tensor_copy(
                            ao_sb[:, g0:g0 + 4 * HD], otg[:])

                wo_pre = {}
                for h in range(HPC):
                    if h == HPC - 1:
                        # prefetch the first two phase-O weight tiles while
                        # the sync queue is otherwise idle
                        for g in (0, 1):
                            wt = wos.tile([128, NC_ * 4 * HD], CDT, tag="wo",
                                          name=f"wo_{g}_0")
                            nc.sync.dma_start(wt[:], wo[g * HPC, :, :])
                            wo_pre[(g, 0)] = wt
                    pend_a = []
                    pend_b = []
                    otg = [None]

                    def run_b(args):
                        hh, bb, jj_, pt_t = args
                        if jj_ % 4 == 3:
                            otg[0] = ops.tile([128, 512], F32, tag="ot",
                                              name=f"ot_{hh}_{bb}_{jj_}")
                        stage2b(hh, bb, jj_, pt_t, otg[0])

                    # q-tiles in descending j: the big tiles lead (hiding the
                    # exp/normalize latency at pipeline fill), the tiny ones
                    # flush the tail just before this head's AllToAll
                    for b in range(B):
                        for j in range(QT - 1, -1, -1):
                            p_t = stage1(h, b, j)
                            pend_a.append((h, b, j, p_t))
                            if len(pend_a) > 2:
                                ha, ba, ja, pa = pend_a.pop(0)
                                pend_b.append((ha, ba, ja,
                                               stage2a(ha, ba, ja, pa)))
                            if len(pend_b) > 2:
                                run_b(pend_b.pop(0))
                    for ha, ba, ja, pa in pend_a:
                        pend_b.append((ha, ba, ja, stage2a(ha, ba, ja, pa)))
                    for args in pend_b:
                        run_b(args)
                    # per-head AllToAll, overlaps later heads' attention;
                    # stores on sync+gpsimd only -- NEVER the scalar queue
                    # (a blocked store there stalls the in-order exp stream)
                    for r in range(NC_):
                        seng = nc.sync if r % 2 == 0 else nc.gpsimd
                        seng.dma_start(
                            a2a_in[h][r, :, :],
                            ao_sb[:, h * ROWS + r * RPC:
                                  h * ROWS + (r + 1) * RPC])
                    nc.gpsimd.collective_compute(
                        "AllToAll", mybir.AluOpType.bypass,
                        ins=[a2a_in[h][:].opt()], outs=[a2a_out[h][:].opt()],
                        replica_groups=[list(range(NC_))],
                    )

            persist_cm.__exit__(None, None, None)

            # ---------------- phase O: output projection ----------------
            # h-MAJOR passes with SBUF-accumulated partials: head h's A2A is
            # only needed ~67us*h into phase O, so even ~100us of inter-core
            # arrival skew on the collectives stays off the critical path.
            # PSUM accumulates over sources s within one (g,h) pass; the
            # cross-h accumulation runs on the otherwise-idle DVE in SBUF.
            NG = DT_ // 4   # 8 od-groups of 4 PSUM banks, double-buffered
            with tc.tile_pool(name="osb", bufs=1) as osb, \
                 tc.tile_pool(name="opp", bufs=2, space="PSUM") as opp:
                # NOTE: each tile allocation must immediately precede its
                # first writer — the released-persist-zone dependency is
                # attached to the next emitted instruction, so reordering
                # allocations against the load/sweep emission corrupts data
                # (measured: NaN / 1e-1 errors with po allocated early).
                ao2 = osb.tile([128, DT_ * RPC], CDT)
                # ALL a2a-output loads stay on the gpsimd queue: they wait on
                # collective completion, and the scheduler may hoist them —
                # on sync/scalar such a blocked DMA stalls the exp stream and
                # the whole machine (measured 4-12us global quiesce)
                for h in range(HPC):
                    for s_ in range(NC_):
                        ct = s_ * HPC + h
                        if h == 0:
                            # head 0 -> the early-loadable constp staging
                            nc.gpsimd.dma_start(
                                a2h0[:, s_ * RPC:(s_ + 1) * RPC],
                                a2a_out[h][s_, :, :])
                        else:
                            nc.gpsimd.dma_start(
                                ao2[:, ct * RPC:(ct + 1) * RPC],
                                a2a_out[h][s_, :, :])
                po = osb.tile([128, DT_ * RPC], F32)   # partial out, 32 od
                for h in range(HPC):
                    for g in range(NG):
                        ops_ = opp.tile([128, 4 * RPC], F32, tag="op",
                                        name=f"op_{g}_{h}")
                        if (g, h) in wo_pre:
                            wo_sb = wo_pre.pop((g, h))
                        else:
                            wo_sb = wos.tile([128, NC_ * 4 * HD], CDT,
                                             tag="wo", name=f"wo_{g}_{h}")
                            # alternate queues: halves load latency and keeps
                            # the first passes off the a2a-store backlog
                            weng = nc.sync if (h * NG + g) % 2 == 0 \
                                else nc.scalar
                            weng.dma_start(wo_sb[:], wo[g * HPC + h, :, :])
                        for s_ in range(NC_):
                            ct = s_ * HPC + h
                            rhs = (a2h0[:, s_ * RPC:(s_ + 1) * RPC]
                                   if h == 0 else
                                   ao2[:, ct * RPC:(ct + 1) * RPC])
                            for i in range(4):
                                w0 = (s_ * 4 + i) * HD
                                nc.tensor.matmul(
                                    ops_[:, i * RPC:(i + 1) * RPC],
                                    wo_sb[:, w0:w0 + HD], rhs,
                                    start=(s_ == 0), stop=(s_ == NC_ - 1))
                        for i in range(4):
                            od = g * 4 + i
                            ps = ops_[:, i * RPC:(i + 1) * RPC]
                            pslice = po[:, od * RPC:(od + 1) * RPC]
                            if h == 0:
                                # first head: plain drain (ACT, near PSUM)
                                nc.scalar.copy(pslice, ps)
                            else:
                                nc.vector.tensor_add(pslice, pslice, ps)
                            if h == HPC - 1:
                                # final: store straight from the partial
                                # buffer, alternating DMA queues (both are
                                # idle by now) to shorten the tail
                                oeng = nc.scalar if i % 2 == 0 else nc.sync
                                oeng.dma_start(
                                    out[od * 128:(od + 1) * 128, :], pslice)

            wos_cm.__exit__(None, None, None)

    nc.compile()
    return nc


def _host_prep(x, wq, wk, wv, wo):
    perm = np.concatenate([np.arange(0, HD, 2), np.arange(1, HD, 2)])
    x2 = np.ascontiguousarray(x.reshape(ROWS, D).T)        # [D, ROWS]
    xT_r = x2.reshape(DT_, 128, ROWS).astype(NP_CDT)

    wq_p = wq.reshape(D, HQ, HD)[:, :, perm].reshape(D, HQ * HD)
    wk_p = wk.reshape(D, HKV, HD)[:, :, perm].reshape(D, HKV * HD)

    # per-core weight shards in sbuf tile layout [128p, d-tile, cols]
    def tile_rows(w):  # [D, C] -> [128, DT_*C] with blocks (d, c)
        Dd, C = w.shape
        return np.ascontiguousarray(
            w.reshape(DT_, 128, C).transpose(1, 0, 2).reshape(128, DT_ * C))

    wq_cores = []
    wk_cores = []
    wv_cores = []
    for c in range(NC_):
        wqc = wq_p[:, c * HPC * HD:(c + 1) * HPC * HD]     # [D, 512]
        wq_cores.append(tile_rows(wqc).astype(NP_CDT))
        wk_cores.append(tile_rows(
            wk_p[:, c * HD:(c + 1) * HD]).astype(NP_CDT))
        wv_cores.append(tile_rows(
            wv[:, c * HD:(c + 1) * HD]).astype(NP_CDT))

    # wo lhsT tiles grouped by (od-group g, head-slot h):
    # wo_t[g*HPC+h, p, (s*4+i)*128+j] = wo[(s*HPC+h)*128+p, (g*4+i)*128+j]
    wo_t = np.ascontiguousarray(
        wo.reshape(NC_, HPC, 128, DT_ // 4, 4, 128)
        .transpose(3, 1, 2, 0, 4, 5)
        .reshape((DT_ // 4) * HPC, 128, NC_ * 4 * 128)).astype(NP_CDT)

    inv = 1.0 / (THETA ** (np.arange(0, HD, 2, dtype=np.float64) / HD))
    ang = np.arange(S, dtype=np.float64)[:, None] * inv[None, :]
    cosT = np.cos(ang).T
    sinT = np.sin(ang).T
    csa = np.concatenate([cosT, cosT], axis=0).astype(np.float32)
    csb = np.concatenate([-sinT, sinT], axis=0).astype(np.float32)

    # transposed causal mask in bf16: consumed as matmul lhsT so that
    # (m^T)^T @ I = m accumulates onto the diagonal score block on PE
    m = np.where(np.arange(HD)[None, :] > np.arange(HD)[:, None],
                 np.float32(-1e9), np.float32(0.0)).T.astype(NP_CDT)
    m = np.ascontiguousarray(m)
    ident = np.eye(128, dtype=np.float32).astype(NP_CDT)
    return (xT_r, wq_cores, wk_cores, wv_cores, wo_t, csa, csb, m, ident)


def kernel(x, wq, wk, wv, wo):
    if "nc" not in _CACHE:
        _CACHE["nc"] = _build()
    nc = _CACHE["nc"]

    xT_r, wq_c, wk_c, wv_c, wo_t, csa, csb, m, ident = _host_prep(
        np.asarray(x, np.float32), np.asarray(wq, np.float32),
        np.asarray(wk, np.float32), np.asarray(wv, np.float32),
        np.asarray(wo, np.float32))

    in_maps = []
    for c in range(NC_):
        in_maps.append({
            "xT": xT_r, "wq": wq_c[c], "wk": wk_c[c], "wv": wv_c[c],
            "wo": wo_t, "csa": csa, "csb": csb, "msk": m, "idn": ident,
        })
    res = run_bass_kernel_spmd(nc, in_maps, core_ids=list(range(NC_)))
    _CACHE["last_results"] = res

    outp = np.empty((ROWS, D), np.float32)
    for c in range(NC_):
        outp[c * RPC:(c + 1) * RPC, :] = res.results[c]["out"].T
    return outp.reshape(B, S, D)

